# revision 7
# baseline (speedup 1.0000x reference)
"""HEPT sparse attention for Trainium2 — 8-core SPMD Bass kernel.

Reference computation (per hash-round r, head h):
  hash q/k via shared projection, argsort, gather into blocks of 128,
  blocked RBF attention: so = exp(-0.5*||q_i-k_j||^2) @ v.

Strategy (v4):
  - Host: bitwise-exact hash + argsort (jax CPU), gather, fp16
    quantization, layout packing.
  - Device (per core, 768 blocks of 128): per block the Gram matrix
    -0.5*||q-k||^2 is ONE K=32 fp16 matmul (27 data rows + squared-norm
    rows hi/lo paired with ones rows).
  - Blocks are processed in groups of 8 = 2 PE row-lanes x 4 quads.
    Gram tiles are [128,1024] f32 = 2 PSUM banks; THREE live slots
    (banks 0-5) so the next group's mm1 never waits on the slot chain.
    mm2 outputs go to dedicated PSUM banks 6/7 (alternating), so the
    PSUM->SBUF output copies are fully decoupled from the Gram slots.
  - Consecutive groups use disjoint PE band pairs (0,1)/(2,3) so their
    row-tiled mm1s run concurrently without sharing PSUM banks.
  - exp() is split across engines: most groups on ScalarE (exact ACT
    exp -> fp16), ~1/3 on VectorE via a Schraudolph bit-trick: the host
    pre-scales those blocks by sqrt(1024*log2(e)) so PSUM holds
    x = 1024*log2(A); the DVE computes int16(rint((x + B) max 0)) whose
    bit pattern IS the fp16 code of A*2^10 (B tuned for min A^2-weighted
    rel err ~1.7% on those groups); v of those blocks is pre-scaled by
    2^-10 to compensate exactly. mm2 consumes the tile bitcast to f16.
  - mm2: so = A @ v; output copied PSUM->SBUF as fp16 on VectorE (the
    ScalarE queue stays a pure exp stream), DMA'd out as fp16.
  - Input DMA is prefetched 3 super-tiles ahead.
"""

import os
from contextlib import ExitStack

import numpy as np

# ---- problem constants (hardcoded; kernel.py must be self-contained) ----
N_HASHES = 3
N_HEADS = 8
PADDED_SIZE = 32768
BLOCK = 128
DIM_PER_HEAD = 24
D_QK = 27
NB = PADDED_SIZE // BLOCK          # 256 blocks per (r,h)
N_CORES = 8
UNITS = N_HASHES * N_HEADS         # 24 independent (r,h) units
UPC = UNITS // N_CORES             # 3 units per core
NBLK = UPC * NB                    # 768 blocks per core
KROWS = 32                         # stacked contraction rows per block
GBLK = 8                           # blocks per group (2 lanes x 4 quads)
N_GROUPS = NBLK // GBLK            # 96 groups per core
SUP_GROUPS = 8                     # groups per super-tile (64 blocks)
N_SUPER = N_GROUPS // SUP_GROUPS   # 12 super-tiles per core

# ---- engine assignment (tunable) ----
# exp of group g: DVE (Schraudolph) when EXP_DVE[g], else ScalarE (exact).
N_DVE = 34
EXP_DVE = [(g * N_DVE) % N_GROUPS < N_DVE for g in range(N_GROUPS)]

# Schraudolph constants: host pre-scales DVE-group q/k stacks by
# SCHRA_S so the PSUM Gram holds x = 1024*log2(A); the DVE computes
# int16(rint((x + B) max 0)) whose bit pattern is the fp16 code of
# A * 2^SCHRA_DELTA (the exponent shift keeps the useful A range away
# from the fp16 subnormal cliff); v of those blocks is pre-scaled by
# 2^-SCHRA_DELTA on the host to compensate exactly.
SCHRA_S = float(np.sqrt(1024.0 / np.log(2.0)))   # sqrt(1024*log2(e))
SCHRA_DELTA = 10
SCHRA_B = 25536.5                  # 15360 + 1024*DELTA + sigma (fitted)

_NC_CACHE = {}


# columns per super-tile in the combined input tensor (fp16):
#   q-stacks 16*128 | k-stacks 16*128 | v 64*24
SUP_Q_COLS = 16 * 128
SUP_V_COLS = 64 * 24
SUP_COLS = 2 * SUP_Q_COLS + SUP_V_COLS
PREFETCH = 3                       # input DMAs issued this many supers ahead


def build_nc(ipool_bufs=PREFETCH + 1, apool_bufs=4, opool_bufs=4):
    """Build the per-core Bass module (same NEFF for all 8 cores).

    PSUM budget: gram pool 3 x [128,1024] f32 = banks 0-5; mm2-output
    pool 2 x [128,512] f32 = banks 6,7 (outputs use cols 0:192 so each
    mm2 stays within one bank). Row-tiled matmuls never share a bank:
    group g uses PE bands {2(g%2), 2(g%2)+1} and its own gram slot.
    """
    import concourse.mybir as mybir
    import concourse.tile as tile
    from concourse.bacc import Bacc

    f16 = mybir.dt.float16
    f32 = mybir.dt.float32
    i16 = mybir.dt.int16

    nc = Bacc()
    im = nc.declare_dram_parameter("in", [128, N_SUPER * SUP_COLS], f16,
                                   isOutput=False)
    om = nc.declare_dram_parameter("out", [128, NBLK * 24], f16,
                                   isOutput=True)

    with tile.TileContext(nc) as tc, ExitStack() as ctx:
        ipool = ctx.enter_context(tc.tile_pool(name="ipool", bufs=ipool_bufs))
        apool = ctx.enter_context(tc.tile_pool(name="apool", bufs=apool_bufs))
        opool = ctx.enter_context(tc.tile_pool(name="opool", bufs=opool_bufs))
        gpool = ctx.enter_context(tc.tile_pool(name="gram", bufs=3,
                                               space="PSUM"))
        opspool = ctx.enter_context(tc.tile_pool(name="ops", bufs=2,
                                                 space="PSUM"))

        def issue_in(s):
            t = ipool.tile([128, SUP_COLS], f16)
            c0 = s * SUP_COLS
            nc.sync.dma_start(out=t, in_=im[:, c0:c0 + SUP_COLS])
            return t

        pending = [issue_in(s) for s in range(min(PREFETCH, N_SUPER))]
        gctx = {}                   # g -> (gram, a_t, v_t)

        def start_group(gl, g, q_t, k_t, v_t):
            gram = gpool.tile([128, 1024], f32)
            boff = 2 * (gl % 2)     # PE band pair and partition offset
            for i in (0, 1):        # lane within group
                for gq in range(4):
                    u = (gl // 2) * 4 + gq      # col-unit within super
                    p0 = 32 * (boff + i)
                    nc.tensor.matmul(
                        out=gram[:, i * 512 + gq * 128:i * 512 + (gq + 1) * 128],
                        lhsT=k_t[p0:p0 + 32, u * 128:(u + 1) * 128],
                        rhs=q_t[p0:p0 + 32, u * 128:(u + 1) * 128],
                        start=True, stop=True,
                        tile_position=(p0, 0),
                    )
            if EXP_DVE[g]:
                a_raw = apool.tile([128, 1024], i16)
                nc.vector.tensor_scalar(
                    out=a_raw, in0=gram,
                    scalar1=SCHRA_B, scalar2=0.0,
                    op0=mybir.AluOpType.add, op1=mybir.AluOpType.max,
                )
                a_t = a_raw[:, :].bitcast(f16)
            else:
                a_t = apool.tile([128, 1024], f16)
                nc.scalar.activation(
                    out=a_t, in_=gram,
                    func=mybir.ActivationFunctionType.Exp,
                )
            gctx[g] = (a_t, v_t)

        def finish_group(g):
            a_t, v_t = gctx.pop(g)
            ops = opspool.tile([128, 512], f32)
            for j in range(GBLK):   # so = A @ v per block
                i, gq = j // 4, j % 4
                b_local = (g % SUP_GROUPS) * GBLK + j
                acol = i * 512 + gq * 128
                nc.tensor.matmul(
                    out=ops[:, j * 24:(j + 1) * 24],
                    lhsT=a_t[:, acol:acol + 128],
                    rhs=v_t[:, b_local * 24:(b_local + 1) * 24],
                    start=True, stop=True,
                )
            o_t = opool.tile([128, GBLK * 24], f16)
            nc.vector.tensor_copy(out=o_t, in_=ops[:, 0:GBLK * 24])
            nc.sync.dma_start(out=om[:, g * 192:(g + 1) * 192], in_=o_t)

        # software-pipelined by one group: mm2/copy/dma of group g-1 are
        # issued after mm1/exp of group g.
        for s in range(N_SUPER):
            in_t = pending.pop(0)
            if s + PREFETCH < N_SUPER:
                pending.append(issue_in(s + PREFETCH))
            q_t = in_t[:, 0:SUP_Q_COLS]
            k_t = in_t[:, SUP_Q_COLS:2 * SUP_Q_COLS]
            v_t = in_t[:, 2 * SUP_Q_COLS:SUP_COLS]

            for gl in range(SUP_GROUPS):
                g = s * SUP_GROUPS + gl
                start_group(gl, g, q_t, k_t, v_t)
                if g > 0:
                    finish_group(g - 1)
        finish_group(N_GROUPS - 1)
    nc.finalize()
    return nc


def _get_nc():
    if "nc" not in _NC_CACHE:
        _NC_CACHE["nc"] = build_nc()
    return _NC_CACHE["nc"]


# ---------------- host-side preparation ----------------

def _sort_indices(query, key, combined_shifts, alpha):
    """Replicate the reference's hash + argsort with jax on CPU.

    Uses the exact same jnp ops the reference uses so the fp32 values
    (and therefore the argsort permutations) match bit-for-bit.
    """
    import jax
    import jax.numpy as jnp

    cpu = jax.devices("cpu")[0]
    with jax.default_device(cpu):
        q = jnp.asarray(query)
        k = jnp.asarray(key)
        al = jnp.asarray(alpha)
        cs_i = jnp.asarray(combined_shifts)
        q_hashed = jnp.einsum('hnd,hdr->rhn', q, al)
        k_hashed = jnp.einsum('hnd,hdr->rhn', k, al)
        max_shift = jnp.maximum(q_hashed.max(-1, keepdims=True),
                                k_hashed.max(-1, keepdims=True))
        min_shift = jnp.minimum(q_hashed.min(-1, keepdims=True),
                                k_hashed.min(-1, keepdims=True))
        hash_shift = max_shift - min_shift
        cs = cs_i.astype(q_hashed.dtype) * hash_shift
        q_pos = np.asarray(jnp.argsort(q_hashed + cs, axis=-1))
        k_pos = np.asarray(jnp.argsort(k_hashed + cs, axis=-1))
    return q_pos, k_pos


def _split16(x):
    hi = x.astype(np.float16)
    lo = (x - hi.astype(np.float32)).astype(np.float16)
    return hi, lo


def _build_stack(s_qk, is_k):
    """(UNITS*NB, 128, 27) f32 -> (UNITS*NB, 32, 128) fp16 stack."""
    nblk = s_qk.shape[0]
    hi = s_qk.astype(np.float16)                       # (b, i, d)
    sqm = -0.5 * np.einsum('bid,bid->bi', hi.astype(np.float32),
                           hi.astype(np.float32))      # (b, i) f32
    sqm = np.maximum(sqm, -64000.0)                    # fp16 headroom guard
    sq_hi, sq_lo = _split16(sqm)
    st = np.zeros((nblk, KROWS, BLOCK), np.float16)
    st[:, :D_QK, :] = hi.transpose(0, 2, 1)            # rows 0-26: x^T
    if is_k:
        st[:, 27, :] = 1.0                             # pair of q's sq rows
        st[:, 28, :] = 1.0
        st[:, 29, :] = sq_hi                           # -0.5*||k||^2 hi
        st[:, 30, :] = sq_lo
    else:
        st[:, 27, :] = sq_hi                           # -0.5*||q||^2 hi
        st[:, 28, :] = sq_lo
        st[:, 29, :] = 1.0                             # pair of k's sq rows
        st[:, 30, :] = 1.0
    return st


def _pack_core(stack_blocks):
    """(768, 32, 128) core stacks -> (128, 12*2048) packed q (or k) plane.

    Within super s, col-unit u = pair*4 + gq (16 units of 128 cols);
    partition band 2*(gl%2)+i (32 rows) holds the stack of block
    b = 64*s + 8*gl + 4*i + gq, where pair = gl//2.
    """
    out = np.empty((128, N_SUPER * SUP_Q_COLS), np.float16)
    sb = stack_blocks.reshape(N_SUPER, SUP_GROUPS, 2, 4, KROWS, BLOCK)
    # index [s, gl, i, gq, row, col] -> band = 2*(gl%2)+i, unit = (gl//2)*4+gq
    sb = sb.reshape(N_SUPER, 4, 2, 2, 4, KROWS, BLOCK)
    # dims: s, pair(gl//2), par(gl%2), i, gq, row, col
    # target: [s][band(par,i), row][unit(pair,gq), col]
    sb = sb.transpose(0, 2, 3, 5, 1, 4, 6)   # s, par, i, row, pair, gq, col
    out[:] = sb.reshape(N_SUPER, 128, SUP_Q_COLS).transpose(1, 0, 2) \
               .reshape(128, N_SUPER * SUP_Q_COLS)
    return out


def _dve_block_scale():
    """Per-block (within a core) scale: SCHRA_S for DVE groups, 1 else."""
    per_group = np.where(np.array(EXP_DVE), SCHRA_S, 1.0)
    return np.repeat(per_group, GBLK).astype(np.float32)   # (NBLK,)


def prepare_in_maps(query, key, value, combined_shifts, alpha):
    query = np.ascontiguousarray(np.asarray(query), dtype=np.float32)
    key = np.ascontiguousarray(np.asarray(key), dtype=np.float32)
    value = np.ascontiguousarray(np.asarray(value), dtype=np.float32)
    combined_shifts = np.asarray(combined_shifts)
    alpha = np.asarray(alpha, dtype=np.float32)

    q_pos, k_pos = _sort_indices(query, key, combined_shifts, alpha)

    h_idx = np.arange(N_HEADS)[None, :, None]
    s_query = query[h_idx, q_pos].reshape(UNITS * NB, BLOCK, D_QK)
    s_key = key[h_idx, k_pos].reshape(UNITS * NB, BLOCK, D_QK)
    s_value = value[h_idx, k_pos].reshape(UNITS * NB, BLOCK, DIM_PER_HEAD)

    # pre-scale the DVE-group blocks so PSUM = 1024*log2(A); v of those
    # blocks absorbs the 2^-SCHRA_DELTA exponent-shift compensation
    bscale = np.tile(_dve_block_scale(), N_CORES)        # (UNITS*NB,)
    s_query = s_query * bscale[:, None, None]
    s_key = s_key * bscale[:, None, None]
    vscale = np.where(bscale > 1.0, 2.0 ** -SCHRA_DELTA, 1.0)
    s_value = s_value * vscale[:, None, None]

    qstack = _build_stack(s_query, is_k=False)
    kstack = _build_stack(s_key, is_k=True)
    v16 = s_value.astype(np.float16)

    in_maps = []
    for c in range(N_CORES):
        b0, b1 = c * NBLK, (c + 1) * NBLK
        qp = _pack_core(qstack[b0:b1])              # [128, 12*2048]
        kp = _pack_core(kstack[b0:b1])
        vp = v16[b0:b1].transpose(1, 0, 2).reshape(128, NBLK * 24)
        combined = np.empty((128, N_SUPER * SUP_COLS), np.float16)
        for s in range(N_SUPER):
            c0 = s * SUP_COLS
            combined[:, c0:c0 + SUP_Q_COLS] = \
                qp[:, s * SUP_Q_COLS:(s + 1) * SUP_Q_COLS]
            combined[:, c0 + SUP_Q_COLS:c0 + 2 * SUP_Q_COLS] = \
                kp[:, s * SUP_Q_COLS:(s + 1) * SUP_Q_COLS]
            combined[:, c0 + 2 * SUP_Q_COLS:c0 + SUP_COLS] = \
                vp[:, s * SUP_V_COLS:(s + 1) * SUP_V_COLS]
        in_maps.append({"in": combined})
    return in_maps


def assemble_output(results):
    """results: list of 8 dicts with 'out' [128, 768*24] f16."""
    out = np.empty((UNITS, NB, BLOCK, DIM_PER_HEAD), np.float32)
    for c in range(N_CORES):
        so = np.asarray(results[c]["out"]).astype(np.float32)
        so = so.reshape(128, NBLK, 24)
        out[c * UPC:(c + 1) * UPC] = (
            so.transpose(1, 0, 2).reshape(UPC, NB, BLOCK, DIM_PER_HEAD))
    return out.reshape(N_HASHES, N_HEADS, NB, BLOCK, DIM_PER_HEAD)


def run(query, key, value, combined_shifts, alpha, trace=False):
    from concourse.bass_utils import run_bass_kernel_spmd

    in_maps = prepare_in_maps(query, key, value, combined_shifts, alpha)
    nc = _get_nc()
    res = run_bass_kernel_spmd(
        nc, in_maps, core_ids=list(range(N_CORES)), trace=trace)
    out = assemble_output(res.results)
    return out, res


def kernel(query, key, value, combined_shifts, alpha):
    out, _ = run(query, key, value, combined_shifts, alpha,
                 trace=bool(int(os.environ.get("HEPT_TRACE", "0"))))
    return out


# revision 10
# speedup vs baseline: 1.5465x; 1.5465x over previous
"""HEPT sparse attention for Trainium2 — 8-core SPMD Bass kernel.

Reference computation (per hash-round r, head h):
  hash q/k via shared projection, argsort, gather into blocks of 128,
  blocked RBF attention: so = exp(-0.5*||q_i-k_j||^2) @ v.

Strategy (v4):
  - Host: bitwise-exact hash + argsort (jax CPU), gather, fp16
    quantization, layout packing.
  - Device (per core, 768 blocks of 128): per block the Gram matrix
    -0.5*||q-k||^2 is ONE K=32 fp16 matmul (27 data rows + squared-norm
    rows hi/lo paired with ones rows).
  - Blocks are processed in groups of 8 = 2 PE row-lanes x 4 quads.
    Gram tiles are [128,1024] f32 = 2 PSUM banks; THREE live slots
    (banks 0-5) so the next group's mm1 never waits on the slot chain.
    mm2 outputs go to dedicated PSUM banks 6/7 (alternating), so the
    PSUM->SBUF output copies are fully decoupled from the Gram slots.
  - Consecutive groups use disjoint PE band pairs (0,1)/(2,3) so their
    row-tiled mm1s run concurrently without sharing PSUM banks.
  - exp() is split across engines: most groups on ScalarE (exact ACT
    exp -> fp16), ~1/3 on VectorE via a Schraudolph bit-trick: the host
    pre-scales those blocks by sqrt(1024*log2(e)) so PSUM holds
    x = 1024*log2(A); the DVE computes int16(rint((x + B) max 0)) whose
    bit pattern IS the fp16 code of A*2^10 (B tuned for min A^2-weighted
    rel err ~1.7% on those groups); v of those blocks is pre-scaled by
    2^-10 to compensate exactly. mm2 consumes the tile bitcast to f16.
  - mm2: so = A @ v; output copied PSUM->SBUF as fp16 on VectorE (the
    ScalarE queue stays a pure exp stream), DMA'd out as fp16.
  - Input DMA is prefetched 3 super-tiles ahead.
"""

import os
from contextlib import ExitStack

import numpy as np

# ---- problem constants (hardcoded; kernel.py must be self-contained) ----
N_HASHES = 3
N_HEADS = 8
PADDED_SIZE = 32768
BLOCK = 128
DIM_PER_HEAD = 24
D_QK = 27
NB = PADDED_SIZE // BLOCK          # 256 blocks per (r,h)
N_CORES = 8
UNITS = N_HASHES * N_HEADS         # 24 independent (r,h) units
UPC = UNITS // N_CORES             # 3 units per core
NBLK = UPC * NB                    # 768 blocks per core
KROWS = 32                         # stacked contraction rows per block
GBLK = 8                           # blocks per group (2 lanes x 4 quads)
N_GROUPS = NBLK // GBLK            # 96 groups per core
SUP_GROUPS = 8                     # groups per super-tile (64 blocks)
N_SUPER = N_GROUPS // SUP_GROUPS   # 12 super-tiles per core

# ---- engine assignment (tunable) ----
# exp of group g: DVE (Schraudolph) when EXP_DVE[g], else ScalarE (exact).
N_DVE = 35
EXP_DVE = [(g * N_DVE) % N_GROUPS < N_DVE for g in range(N_GROUPS)]

# Schraudolph constants: host pre-scales DVE-group q/k stacks by
# SCHRA_S so the PSUM Gram holds x = 1024*log2(A); the DVE computes
# int16(rint((x + B) max 0)) whose bit pattern is the fp16 code of
# A * 2^SCHRA_DELTA (the exponent shift keeps the useful A range away
# from the fp16 subnormal cliff); v of those blocks is pre-scaled by
# 2^-SCHRA_DELTA on the host to compensate exactly.
SCHRA_S = float(np.sqrt(1024.0 / np.log(2.0)))   # sqrt(1024*log2(e))
SCHRA_DELTA = 10
SCHRA_B = 25536.5                  # 15360 + 1024*DELTA + sigma (fitted)

_NC_CACHE = {}


# columns per super-tile in the combined input tensor (fp16):
#   q-stacks 16*128 | k-stacks 16*128 | v 64*24
SUP_Q_COLS = 16 * 128
SUP_V_COLS = 64 * 24
SUP_COLS = 2 * SUP_Q_COLS + SUP_V_COLS
PREFETCH = 3                       # input DMAs issued this many supers ahead


def build_nc(ipool_bufs=PREFETCH + 1, apool_bufs=4, opool_bufs=4):
    """Build the per-core Bass module (same NEFF for all 8 cores).

    PSUM budget: gram pool 3 x [128,1024] f32 = banks 0-5; mm2-output
    pool 2 x [128,512] f32 = banks 6,7 (outputs use cols 0:192 so each
    mm2 stays within one bank). Row-tiled matmuls never share a bank:
    group g uses PE bands {2(g%2), 2(g%2)+1} and its own gram slot.
    """
    import concourse.mybir as mybir
    import concourse.tile as tile
    from concourse.bacc import Bacc

    f16 = mybir.dt.float16
    f32 = mybir.dt.float32
    i16 = mybir.dt.int16

    nc = Bacc()
    im = nc.declare_dram_parameter("in", [128, N_SUPER * SUP_COLS], f16,
                                   isOutput=False)
    om = nc.declare_dram_parameter("out", [128, NBLK * 24], f16,
                                   isOutput=True)

    with tile.TileContext(nc) as tc, ExitStack() as ctx:
        ipool = ctx.enter_context(tc.tile_pool(name="ipool", bufs=ipool_bufs))
        apool = ctx.enter_context(tc.tile_pool(name="apool", bufs=apool_bufs))
        opool = ctx.enter_context(tc.tile_pool(name="opool", bufs=opool_bufs))
        gpool = ctx.enter_context(tc.tile_pool(name="gram", bufs=3,
                                               space="PSUM"))
        opspool = ctx.enter_context(tc.tile_pool(name="ops", bufs=2,
                                                 space="PSUM"))

        def issue_in(s):
            t = ipool.tile([128, SUP_COLS], f16)
            c0 = s * SUP_COLS
            nc.sync.dma_start(out=t, in_=im[:, c0:c0 + SUP_COLS])
            return t

        pending = [issue_in(s) for s in range(min(PREFETCH, N_SUPER))]
        gctx = {}                   # g -> (gram, a_t, v_t)

        def start_group(gl, g, q_t, k_t, v_t):
            gram = gpool.tile([128, 1024], f32)
            boff = 2 * (gl % 2)     # PE band pair and partition offset
            for i in (0, 1):        # lane within group
                for gq in range(4):
                    u = (gl // 2) * 4 + gq      # col-unit within super
                    p0 = 32 * (boff + i)
                    nc.tensor.matmul(
                        out=gram[:, i * 512 + gq * 128:i * 512 + (gq + 1) * 128],
                        lhsT=k_t[p0:p0 + 32, u * 128:(u + 1) * 128],
                        rhs=q_t[p0:p0 + 32, u * 128:(u + 1) * 128],
                        start=True, stop=True,
                        tile_position=(p0, 0),
                    )
            if EXP_DVE[g]:
                a_raw = apool.tile([128, 1024], i16)
                nc.vector.tensor_scalar(
                    out=a_raw, in0=gram,
                    scalar1=SCHRA_B, scalar2=0.0,
                    op0=mybir.AluOpType.add, op1=mybir.AluOpType.max,
                )
                a_t = a_raw[:, :].bitcast(f16)
            else:
                a_t = apool.tile([128, 1024], f16)
                nc.scalar.activation(
                    out=a_t, in_=gram,
                    func=mybir.ActivationFunctionType.Exp,
                )
            gctx[g] = (a_t, v_t)

        fctx = {"ops": None, "o_t": None}

        def finish_group(g):
            a_t, v_t = gctx.pop(g)
            # mm2 outputs of a PAIR of groups share one 1-bank psum
            # tile; copies run once per pair; DMA once per 4 groups.
            if g % 2 == 0:
                fctx["ops"] = opspool.tile([128, 512], f32, name="ops")
            if g % 4 == 0:
                fctx["o_t"] = opool.tile([128, 4 * GBLK * 24], f16, name="o_t")
            ops = fctx["ops"]
            half = (g % 2) * GBLK * 24
            for j in range(GBLK):   # so = A @ v per block
                i, gq = j // 4, j % 4
                b_local = (g % SUP_GROUPS) * GBLK + j
                acol = i * 512 + gq * 128
                nc.tensor.matmul(
                    out=ops[:, half + j * 24:half + (j + 1) * 24],
                    lhsT=a_t[:, acol:acol + 128],
                    rhs=v_t[:, b_local * 24:(b_local + 1) * 24],
                    start=True, stop=True,
                )
            if g % 2 == 1:
                o_t = fctx["o_t"]
                qoff = (g % 4 // 2) * 2 * GBLK * 24
                nc.vector.tensor_copy(
                    out=o_t[:, qoff:qoff + 2 * GBLK * 24],
                    in_=ops[:, 0:2 * GBLK * 24])
            if g % 4 == 3:
                g0 = g - 3
                nc.sync.dma_start(
                    out=om[:, g0 * 192:(g0 + 4) * 192], in_=fctx["o_t"])

        # software-pipelined by one group: mm2/copy/dma of group g-1 are
        # issued after mm1/exp of group g.
        for s in range(N_SUPER):
            in_t = pending.pop(0)
            if s + PREFETCH < N_SUPER:
                pending.append(issue_in(s + PREFETCH))
            q_t = in_t[:, 0:SUP_Q_COLS]
            k_t = in_t[:, SUP_Q_COLS:2 * SUP_Q_COLS]
            v_t = in_t[:, 2 * SUP_Q_COLS:SUP_COLS]

            for gl in range(SUP_GROUPS):
                g = s * SUP_GROUPS + gl
                start_group(gl, g, q_t, k_t, v_t)
                if g > 0:
                    finish_group(g - 1)
        finish_group(N_GROUPS - 1)
    nc.finalize()
    return nc


def _get_nc():
    if "nc" not in _NC_CACHE:
        _NC_CACHE["nc"] = build_nc()
    return _NC_CACHE["nc"]


# ---------------- host-side preparation ----------------

def _sort_indices(query, key, combined_shifts, alpha):
    """Replicate the reference's hash + argsort with jax on CPU.

    Uses the exact same jnp ops the reference uses so the fp32 values
    (and therefore the argsort permutations) match bit-for-bit.
    """
    import jax
    import jax.numpy as jnp

    cpu = jax.devices("cpu")[0]
    with jax.default_device(cpu):
        q = jnp.asarray(query)
        k = jnp.asarray(key)
        al = jnp.asarray(alpha)
        cs_i = jnp.asarray(combined_shifts)
        q_hashed = jnp.einsum('hnd,hdr->rhn', q, al)
        k_hashed = jnp.einsum('hnd,hdr->rhn', k, al)
        max_shift = jnp.maximum(q_hashed.max(-1, keepdims=True),
                                k_hashed.max(-1, keepdims=True))
        min_shift = jnp.minimum(q_hashed.min(-1, keepdims=True),
                                k_hashed.min(-1, keepdims=True))
        hash_shift = max_shift - min_shift
        cs = cs_i.astype(q_hashed.dtype) * hash_shift
        q_pos = np.asarray(jnp.argsort(q_hashed + cs, axis=-1))
        k_pos = np.asarray(jnp.argsort(k_hashed + cs, axis=-1))
    return q_pos, k_pos


def _split16(x):
    hi = x.astype(np.float16)
    lo = (x - hi.astype(np.float32)).astype(np.float16)
    return hi, lo


def _build_stack(s_qk, is_k):
    """(UNITS*NB, 128, 27) f32 -> (UNITS*NB, 32, 128) fp16 stack."""
    nblk = s_qk.shape[0]
    hi = s_qk.astype(np.float16)                       # (b, i, d)
    sqm = -0.5 * np.einsum('bid,bid->bi', hi.astype(np.float32),
                           hi.astype(np.float32))      # (b, i) f32
    sqm = np.maximum(sqm, -64000.0)                    # fp16 headroom guard
    sq_hi, sq_lo = _split16(sqm)
    st = np.zeros((nblk, KROWS, BLOCK), np.float16)
    st[:, :D_QK, :] = hi.transpose(0, 2, 1)            # rows 0-26: x^T
    if is_k:
        st[:, 27, :] = 1.0                             # pair of q's sq rows
        st[:, 28, :] = 1.0
        st[:, 29, :] = sq_hi                           # -0.5*||k||^2 hi
        st[:, 30, :] = sq_lo
    else:
        st[:, 27, :] = sq_hi                           # -0.5*||q||^2 hi
        st[:, 28, :] = sq_lo
        st[:, 29, :] = 1.0                             # pair of k's sq rows
        st[:, 30, :] = 1.0
    return st


def _pack_core(stack_blocks):
    """(768, 32, 128) core stacks -> (128, 12*2048) packed q (or k) plane.

    Within super s, col-unit u = pair*4 + gq (16 units of 128 cols);
    partition band 2*(gl%2)+i (32 rows) holds the stack of block
    b = 64*s + 8*gl + 4*i + gq, where pair = gl//2.
    """
    out = np.empty((128, N_SUPER * SUP_Q_COLS), np.float16)
    sb = stack_blocks.reshape(N_SUPER, SUP_GROUPS, 2, 4, KROWS, BLOCK)
    # index [s, gl, i, gq, row, col] -> band = 2*(gl%2)+i, unit = (gl//2)*4+gq
    sb = sb.reshape(N_SUPER, 4, 2, 2, 4, KROWS, BLOCK)
    # dims: s, pair(gl//2), par(gl%2), i, gq, row, col
    # target: [s][band(par,i), row][unit(pair,gq), col]
    sb = sb.transpose(0, 2, 3, 5, 1, 4, 6)   # s, par, i, row, pair, gq, col
    out[:] = sb.reshape(N_SUPER, 128, SUP_Q_COLS).transpose(1, 0, 2) \
               .reshape(128, N_SUPER * SUP_Q_COLS)
    return out


def _dve_block_scale():
    """Per-block (within a core) scale: SCHRA_S for DVE groups, 1 else."""
    per_group = np.where(np.array(EXP_DVE), SCHRA_S, 1.0)
    return np.repeat(per_group, GBLK).astype(np.float32)   # (NBLK,)


def prepare_in_maps(query, key, value, combined_shifts, alpha):
    query = np.ascontiguousarray(np.asarray(query), dtype=np.float32)
    key = np.ascontiguousarray(np.asarray(key), dtype=np.float32)
    value = np.ascontiguousarray(np.asarray(value), dtype=np.float32)
    combined_shifts = np.asarray(combined_shifts)
    alpha = np.asarray(alpha, dtype=np.float32)

    q_pos, k_pos = _sort_indices(query, key, combined_shifts, alpha)

    h_idx = np.arange(N_HEADS)[None, :, None]
    s_query = query[h_idx, q_pos].reshape(UNITS * NB, BLOCK, D_QK)
    s_key = key[h_idx, k_pos].reshape(UNITS * NB, BLOCK, D_QK)
    s_value = value[h_idx, k_pos].reshape(UNITS * NB, BLOCK, DIM_PER_HEAD)

    # pre-scale the DVE-group blocks so PSUM = 1024*log2(A); v of those
    # blocks absorbs the 2^-SCHRA_DELTA exponent-shift compensation
    bscale = np.tile(_dve_block_scale(), N_CORES)        # (UNITS*NB,)
    s_query = s_query * bscale[:, None, None]
    s_key = s_key * bscale[:, None, None]
    vscale = np.where(bscale > 1.0, 2.0 ** -SCHRA_DELTA, 1.0)
    s_value = s_value * vscale[:, None, None]

    qstack = _build_stack(s_query, is_k=False)
    kstack = _build_stack(s_key, is_k=True)
    v16 = s_value.astype(np.float16)

    in_maps = []
    for c in range(N_CORES):
        b0, b1 = c * NBLK, (c + 1) * NBLK
        qp = _pack_core(qstack[b0:b1])              # [128, 12*2048]
        kp = _pack_core(kstack[b0:b1])
        vp = v16[b0:b1].transpose(1, 0, 2).reshape(128, NBLK * 24)
        combined = np.empty((128, N_SUPER * SUP_COLS), np.float16)
        for s in range(N_SUPER):
            c0 = s * SUP_COLS
            combined[:, c0:c0 + SUP_Q_COLS] = \
                qp[:, s * SUP_Q_COLS:(s + 1) * SUP_Q_COLS]
            combined[:, c0 + SUP_Q_COLS:c0 + 2 * SUP_Q_COLS] = \
                kp[:, s * SUP_Q_COLS:(s + 1) * SUP_Q_COLS]
            combined[:, c0 + 2 * SUP_Q_COLS:c0 + SUP_COLS] = \
                vp[:, s * SUP_V_COLS:(s + 1) * SUP_V_COLS]
        in_maps.append({"in": combined})
    return in_maps


def assemble_output(results):
    """results: list of 8 dicts with 'out' [128, 768*24] f16."""
    out = np.empty((UNITS, NB, BLOCK, DIM_PER_HEAD), np.float32)
    for c in range(N_CORES):
        so = np.asarray(results[c]["out"]).astype(np.float32)
        so = so.reshape(128, NBLK, 24)
        out[c * UPC:(c + 1) * UPC] = (
            so.transpose(1, 0, 2).reshape(UPC, NB, BLOCK, DIM_PER_HEAD))
    return out.reshape(N_HASHES, N_HEADS, NB, BLOCK, DIM_PER_HEAD)


def run(query, key, value, combined_shifts, alpha, trace=False):
    from concourse.bass_utils import run_bass_kernel_spmd

    in_maps = prepare_in_maps(query, key, value, combined_shifts, alpha)
    nc = _get_nc()
    res = run_bass_kernel_spmd(
        nc, in_maps, core_ids=list(range(N_CORES)), trace=trace)
    out = assemble_output(res.results)
    return out, res


def kernel(query, key, value, combined_shifts, alpha):
    out, _ = run(query, key, value, combined_shifts, alpha,
                 trace=bool(int(os.environ.get("HEPT_TRACE", "0"))))
    return out
